# revision 11
# baseline (speedup 1.0000x reference)
"""MoE GroupedExperts kernel for 8 TRN2 NeuronCores.

Expert-parallel: expert e's tokens + weights go to core e. Tokens are
pre-sorted by expert, so routing is host-side slicing. Each core runs a
SwiGLU MLP: o = (silu(x @ gate) * (x @ up)) @ down.

Weights are repacked host-side into partition-major, chunk-major DRAM
layout so every weight DMA is 128 descriptors x 4KB contiguous runs --
this keeps effective HBM bandwidth near the ~390 GB/s cap, which paces
the first third of this kernel. x and the first gate chunk are packed
into combined head transfers so the PE's first real matmul waits on one
fat DMA instead of a chain of thin ones. The PE is kept busy from
engine release with throwaway matmuls so the HAM clock gate reaches
2.4 GHz before the real GEMM stream starts.
"""

import sys

if "/opt/trn_rl_repo" not in sys.path:
    sys.path.insert(0, "/opt/trn_rl_repo")

import numpy as np

BF16 = np.float16
E = 8
DIM = 1024
HID = 2048
N_CORES = 8
CMAX_BLOCK = 512  # max tokens per device invocation (PSUM free-dim limit)

_cache = {}


def _build(cpad: int):
    """Build + compile the per-core kernel for cpad tokens per expert."""
    from concourse import bacc
    import concourse.tile as tile
    import concourse.mybir as mybir

    f32 = mybir.dt.float32
    bf16 = mybir.dt.float16  # fp16: same PE rate as bf16, 3 more mantissa bits

    KC = DIM // 128   # 8 k-chunks for gate/up contraction
    KH = HID // 128   # 16 k-chunks for down contraction
    NH = HID // 128   # 16 hid slices of the gate/up output
    NTOK = cpad // 128  # token tiles

    # Pair hid slices so one PSUM bank (512 fp32/partition) holds a
    # whole silu/mul group.
    PAIR = max(1, min(NH, 512 // cpad))
    W = PAIR * 128        # hid cols per chunk == silu group width
    NG = HID // W         # hid groups / weight chunks per matrix
    NDC = DIM // 512      # down-proj output column halves
    NKG = 4               # dw k-chunk groups per dc piece
    KGS = KH // NKG
    KSP = KC // PAIR      # x k-chunks per head transfer
    XW = KSP * cpad       # x cols per head transfer (per partition)
    GW0 = KC * 128        # g00/u00 cols per j chunk

    # Warm-up matmuls bridge PE release to first-operand arrival
    # (~release + 4us): cold MMs at ~107ns each, slightly overshooting
    # -- a >=3.4us idle gap before the real stream would re-throttle
    # the HAM clock gate to 1.2 GHz, which costs far more.
    N_WARMUP = 34

    nc = bacc.Bacc("TRN2", target_bir_lowering=False, debug=False)
    # Head transfers: xg[j] = [x k-group j | gate00 j-chunk] packed into
    # one [128, XW+GW0] fp16 DRAM tensor -> a single DMA with 4KB-per-
    # partition descriptors covers everything the first matmuls need.
    # uu = both up00 j-chunks packed the same way.
    xg_d = [
        nc.dram_tensor(f"xg{j}", [128, XW + GW0], bf16, kind="ExternalInput")
        for j in range(PAIR)
    ]
    uu_d = nc.dram_tensor("uu", [128, PAIR * GW0], bf16, kind="ExternalInput")
    gw_d = nc.dram_tensor("gw", [NG - 1, 128, KC, W], bf16, kind="ExternalInput")
    uw_d = nc.dram_tensor("uw", [NG - 1, 128, KC, W], bf16, kind="ExternalInput")
    dw_d = nc.dram_tensor("dw", [NDC, NKG, 128, KGS, 512], bf16, kind="ExternalInput")
    o_d = nc.dram_tensor("o", [cpad, DIM], bf16, kind="ExternalOutput")

    with tile.TileContext(nc) as tc:
        with (
            tc.tile_pool(name="sb", bufs=1) as sb,
            tc.tile_pool(name="stmp", bufs=2) as stmp_pool,
            tc.tile_pool(name="ht", bufs=NG) as ht_pool,
            tc.tile_pool(name="outp", bufs=2) as out_pool,
            tc.tile_pool(name="psW", bufs=1, space="PSUM") as psW,
            tc.tile_pool(name="psA", bufs=2, space="PSUM") as psA,
            tc.tile_pool(name="psB", bufs=2, space="PSUM") as psB,
            tc.tile_pool(name="psO", bufs=3, space="PSUM") as psO,
        ):
            xg_s = [sb.tile([128, XW + GW0], bf16, name=f"xg{j}_s")
                    for j in range(PAIR)]
            uu_s = sb.tile([128, PAIR * GW0], bf16)
            gw_s = sb.tile([128, NG - 1, KC, W], bf16)
            uw_s = sb.tile([128, NG - 1, KC, W], bf16)
            dw_s = sb.tile([128, NDC, NKG, KGS, 512], bf16)
            wu = sb.tile([128, 128], bf16)

            # PE warm-up spin; memset on gpsimd (released earliest).
            nc.gpsimd.memset(wu[:], 0.0)
            pw = psW.tile([128, 128], f32)
            for i in range(N_WARMUP):
                nc.tensor.matmul(pw[:], wu[:], wu[:], start=True, stop=True,
                                 skip_group_check=True)

            # All inbound DMA rides the sync HWDGE ring in exact
            # consumption order; outputs ride scalar. Every transfer is
            # 128 fat descriptors (>=4KB) thanks to the host packing.
            for j in range(PAIR):
                nc.sync.dma_start(xg_s[j][:], xg_d[j].ap())
            nc.sync.dma_start(uu_s[:], uu_d.ap())
            for c in range(NG - 1):
                nc.sync.dma_start(gw_s[:, c], gw_d.ap()[c])
                nc.sync.dma_start(uw_s[:, c], uw_d.ap()[c])
            for dc in range(NDC):
                for kg in range(NKG):
                    nc.sync.dma_start(dw_s[:, dc, kg], dw_d.ap()[dc, kg])

            # Operand slices. x chunk k lives in head transfer k//KSP;
            # group-0 gate chunk j lives at the tail of head transfer j;
            # group-0 up chunks live in uu.
            def xsl(k):
                t = xg_s[k // KSP]
                o = (k % KSP) * cpad
                return t[:, o:o + cpad]

            def gsl(g, j, k):
                if g == 0:
                    return xg_s[j][:, XW + k * 128:XW + (k + 1) * 128]
                return gw_s[:, g - 1, k, j * 128:(j + 1) * 128]

            def usl(g, j, k):
                if g == 0:
                    return uu_s[:, j * GW0 + k * 128:j * GW0 + (k + 1) * 128]
                return uw_s[:, g - 1, k, j * 128:(j + 1) * 128]

            # Gate/up grouped GEMMs; h produced in [hid, tok] layout,
            # PAIR hid slices per PSUM bank side by side. Silu is issued
            # right after the gate group so ACT overlaps the up MMs.
            ht = []
            for g in range(NG):
                pg = psA.tile([128, PAIR, cpad], f32, tag="pg")
                pu = psB.tile([128, PAIR, cpad], f32, tag="pu")
                # Group 0 is ordered by head transfer: (j0,k0-3) runs
                # off the first head DMA alone; (j1,k0-3)+(j0..j1,k4-7)
                # follow the second. start=True clears the WHOLE psum
                # bank's has_written bits, so with j0/j1 interleaved
                # mid-accumulation only the bank's very first matmul may
                # carry start -- later first-writes rely on per-element
                # has_written=0 to overwrite, then accumulate.
                if g == 0:
                    jk = [(j, k) for kh in range(PAIR) for j in range(PAIR)
                          for k in range(kh * KSP, kh * KSP + KSP)]
                else:
                    jk = [(j, k) for j in range(PAIR) for k in range(KC)]
                for idx, (j, k) in enumerate(jk):
                    nc.tensor.matmul(
                        pg[:, j, :], gsl(g, j, k), xsl(k),
                        start=(idx == 0), stop=(k == KC - 1),
                        skip_group_check=True,
                    )
                stmp = stmp_pool.tile([128, PAIR, cpad], f32, tag="stmp")
                nc.scalar.activation(
                    stmp[:], pg[:], mybir.ActivationFunctionType.Silu
                )
                for idx, (j, k) in enumerate(jk):
                    nc.tensor.matmul(
                        pu[:, j, :], usl(g, j, k), xsl(k),
                        start=(idx == 0), stop=(k == KC - 1),
                        skip_group_check=True,
                    )
                ht_t = ht_pool.tile([128, PAIR, cpad], bf16, tag="ht")
                nc.vector.tensor_mul(ht_t[:], stmp[:], pu[:])
                ht.append(ht_t)

            # Down projection: o[tok, dim] = h @ down, dc-outer so each
            # 512-col output piece is copied + DMA'd while the PE works
            # on the next piece (streams the output, shortens the tail).
            for dc in range(NDC):
                for tok in range(NTOK):
                    t0, t1 = tok * 128, (tok + 1) * 128
                    last = (dc == NDC - 1 and tok == NTOK - 1)
                    # The final piece is computed in two 256-col halves
                    # (each in its OWN psum tile, so half-b's matmuls
                    # don't serialize behind half-a's copy) -- the first
                    # half's copy + DMA + completion receipt overlap the
                    # second half's matmuls, shortening the kernel tail.
                    halves = ((0, 256), (256, 512)) if last else ((0, 512),)
                    for h0, h1 in halves:
                        po = psO.tile([128, h1 - h0], f32, tag="po")
                        for kg in range(NKG):
                            for k2 in range(KGS):
                                kk = kg * KGS + k2
                                nc.tensor.matmul(
                                    po[:],
                                    ht[kk // PAIR][:, kk % PAIR, t0:t1],
                                    dw_s[:, dc, kg, k2, h0:h1],
                                    start=(kk == 0), stop=(kk == KH - 1),
                                    skip_group_check=True,
                                )
                        out_s = out_pool.tile([128, h1 - h0], bf16, tag="out")
                        final = last and h0 > 0
                        # Alternate copy engines so PSUM->SBUF evacuation
                        # of piece i overlaps piece i+1's matmuls. The
                        # FINAL half goes ACT-copy + sync-ring DMA: ACT's
                        # 256-col copy is ~2x faster than DVE's, and the
                        # sync ring (idle after the weight stream) runs
                        # its descriptor gen in parallel with scalar's
                        # previous output DMA -- shortens the tail.
                        if final:
                            nc.scalar.copy(out_s[:], po[:])
                            nc.sync.dma_start(
                                o_d[t0:t1, dc * 512 + h0:dc * 512 + h1],
                                out_s[:],
                            )
                            continue
                        if (dc * NTOK + tok + (h0 > 0)) % 2 == 0:
                            nc.vector.tensor_copy(out_s[:], po[:])
                        else:
                            nc.scalar.copy(out_s[:], po[:])
                        nc.scalar.dma_start(
                            o_d[t0:t1, dc * 512 + h0:dc * 512 + h1], out_s[:]
                        )

    nc.compile()
    return nc


def _get_nc(cpad: int):
    if cpad not in _cache:
        _cache[cpad] = _build(cpad)
    return _cache[cpad]


def _pack_weights(gate, up, down, cpad):
    """Repack one expert's fp16 weights into the chunk-major DRAM layout."""
    KC = DIM // 128
    NH = HID // 128
    KH = HID // 128
    PAIR = max(1, min(NH, 512 // cpad))
    W = PAIR * 128
    NG = HID // W
    NDC = DIM // 512
    NKG = 4
    KGS = KH // NKG
    # group 0 split into PAIR 128-col j-chunks, flattened per partition;
    # groups 1.. as W-col chunks
    gr = gate.reshape(KC, 128, NG, W)
    ur = up.reshape(KC, 128, NG, W)
    # [PAIR, 128, KC*128]: partition-major flat j-chunks of group 0
    g0 = np.ascontiguousarray(
        gr[:, :, 0].reshape(KC, 128, PAIR, 128).transpose(2, 1, 0, 3)
    ).reshape(PAIR, 128, KC * 128)
    u0 = np.ascontiguousarray(
        ur[:, :, 0].reshape(KC, 128, PAIR, 128).transpose(2, 1, 0, 3)
    ).reshape(PAIR, 128, KC * 128)
    uu = np.ascontiguousarray(u0.transpose(1, 0, 2).reshape(128, PAIR * KC * 128))
    gw = np.ascontiguousarray(gr[:, :, 1:].transpose(2, 1, 0, 3))
    uw = np.ascontiguousarray(ur[:, :, 1:].transpose(2, 1, 0, 3))
    dw = np.ascontiguousarray(
        down.reshape(NKG, KGS, 128, NDC, 512).transpose(3, 0, 2, 1, 4))
    return g0, uu, gw, uw, dw


def _run_block(nc, xg_blocks, weights, collect):
    """One SPMD invocation: xg_blocks[e] is a list of PAIR [128, XW+GW0]
    packed head arrays."""
    from concourse.bass_utils import run_bass_kernel_spmd

    in_maps = []
    for e in range(E):
        g0, uu, gw, uw, dw = weights[e]
        m = {"uu": uu, "gw": gw, "uw": uw, "dw": dw}
        for j, xg in enumerate(xg_blocks[e]):
            m[f"xg{j}"] = xg
        in_maps.append(m)
    kwargs = {} if collect is None else dict(collect.get("run_kwargs") or {})
    res = run_bass_kernel_spmd(nc, in_maps, core_ids=list(range(N_CORES)), **kwargs)
    if collect is not None:
        collect.setdefault("results", []).append(res)
    return [res.results[e]["o"] for e in range(E)]


def kernel(x, counts, gate_proj, up_proj, down_proj, _collect=None):
    x = np.ascontiguousarray(np.asarray(x, dtype=np.float32))
    counts = np.asarray(counts, dtype=np.int32)
    gate_proj = np.asarray(gate_proj, dtype=np.float32).astype(BF16)
    up_proj = np.asarray(up_proj, dtype=np.float32).astype(BF16)
    down_proj = np.asarray(down_proj, dtype=np.float32).astype(BF16)

    T = x.shape[0]
    offs = np.concatenate([[0], np.cumsum(counts)]).astype(np.int64)
    cmax = int(counts.max()) if counts.size else 128

    n_blocks = max(1, -(-cmax // CMAX_BLOCK))
    if n_blocks == 1:
        cpad = max(128, -(-cmax // 128) * 128)
    else:
        cpad = CMAX_BLOCK

    KC = DIM // 128
    NH = HID // 128
    PAIR = max(1, min(NH, 512 // cpad))
    KSP = KC // PAIR
    nc = _get_nc(cpad)
    weights = [
        _pack_weights(gate_proj[e], up_proj[e], down_proj[e], cpad)
        for e in range(E)
    ]

    out = np.empty((T, DIM), dtype=np.float32)  # o arrives fp16, upcast here
    for b in range(n_blocks):
        xg_blocks = []
        spans = []
        for e in range(E):
            c = int(counts[e])
            s0 = min(b * cpad, c)
            s1 = min((b + 1) * cpad, c)
            xe = x[offs[e] + s0:offs[e] + s1]
            if xe.shape[0] < cpad:
                xe = np.concatenate(
                    [xe, np.zeros((cpad - xe.shape[0], DIM), np.float32)], axis=0
                )
            # [cpad, DIM] -> [128, KC, cpad] partition-major, then pack
            # each x k-group with its group-0 gate j-chunk into one
            # [128, KSP*cpad + KC*128] head array (4KB+ descriptors).
            xt = xe.astype(BF16).reshape(cpad, KC, 128).transpose(2, 1, 0)
            g0 = weights[e][0]
            xg = [
                np.ascontiguousarray(np.concatenate(
                    [xt[:, j * KSP:(j + 1) * KSP].reshape(128, KSP * cpad),
                     g0[j]], axis=1))
                for j in range(PAIR)
            ]
            xg_blocks.append(xg)
            spans.append((s0, s1))
        outs = _run_block(nc, xg_blocks, weights, _collect)
        for e in range(E):
            s0, s1 = spans[e]
            if s1 > s0:
                out[offs[e] + s0:offs[e] + s1] = outs[e][: s1 - s0]
    return out
